# revision 21
# baseline (speedup 1.0000x reference)
"""Trainium2 Bass kernel for nn_FAA (Fourier-argmax alignment).

Per sample (7x7 image): rfft2 magnitudes -> argmax over 27 frequency bins
(weighted by rho) -> rotation angle theta -> bilinear grid-sample rotation.

Device pipeline (2 launches, 8 cores, batch sharded):
  Launch 1 (decision): FFT as matmul (fp16 hi/lo, fp32-accurate), square,
    |F|^2*rho^2 per bin, argmax via mantissa-packed segmented max-reduce.
  Host glue: bin -> unique-angle class, class-sort layout (counting sort).
  Launch 2 (apply): per-class rotation matrices as dense matmuls over the
    class-sorted stream (fp16 hi/lo, fp32-accurate).
"""

import os
import sys

import numpy as np

sys.path.insert(0, "/opt/trn_rl_repo")

import concourse.bacc as bacc
import concourse.tile as tile
from concourse import mybir
from concourse.bass_utils import run_bass_kernel_spmd

F32 = mybir.dt.float32
F16 = mybir.dt.float16
U32 = mybir.dt.uint32

H = W = 7
HW = 49
B = 524288
N_CORES = 8
PER = B // N_CORES  # 65536
GRP = 1024  # samples per launch-1 group (2 subtiles of 512)
N_GRP = PER // GRP  # 64
STAGE_GROUPS = 8  # groups per Mx staging DMA
TILE = 512  # samples per launch-2 matmul tile

LAST_EXEC_NS = {"decision": None, "apply": None}
_TRACE = os.environ.get("FAA_TRACE", "0") == "1"

# ---------------------------------------------------------------------------
# constants (pure math, replicates the reference module exactly)
# ---------------------------------------------------------------------------


def _freq_constants():
    h_shift = np.roll(np.arange(H) - H // 2, H // 2)
    w_shift = np.concatenate([np.arange(W // 2 + 1)[: W // 2], np.array([-(W // 2)])])
    y, xg = np.meshgrid(h_shift, w_shift, indexing="ij")
    rho = np.sqrt((xg**2 + y**2).astype(np.float32))
    theta = np.arctan2(y.astype(np.float32), xg.astype(np.float32))
    theta = (theta + 2.0 * np.pi) % (2.0 * np.pi)
    valid = np.flatnonzero((rho > 1e-8).ravel())
    return valid, theta.ravel()[valid].astype(np.float32), rho.ravel()[valid].astype(
        np.float32
    )


VALID_IDX, VALID_THETAS, VALID_RHOS = _freq_constants()
NBINS = len(VALID_IDX)  # 27


def _fourier_mats():
    C = np.zeros((NBINS, HW), dtype=np.float64)
    S = np.zeros((NBINS, HW), dtype=np.float64)
    for j, flat in enumerate(VALID_IDX):
        u, v = divmod(int(flat), W // 2 + 1)
        for h in range(H):
            for w in range(W):
                ang = 2.0 * np.pi * (u * h + v * w) / 7.0
                C[j, h * W + w] = np.cos(ang) / 7.0
                S[j, h * W + w] = -np.sin(ang) / 7.0
    return C.astype(np.float32), S.astype(np.float32)


CMAT, SMAT = _fourier_mats()

_ys = np.linspace(-1.0, 1.0, H, dtype=np.float32)
_xs = np.linspace(-1.0, 1.0, W, dtype=np.float32)
GY, GX = np.meshgrid(_ys, _xs, indexing="ij")[0], np.meshgrid(_ys, _xs, indexing="ij")[
    1
]


def _rotation_matrix(theta):
    c = np.float32(np.cos(theta))
    s = np.float32(np.sin(theta))
    sx = GX * c - GY * s
    sy = GX * s + GY * c
    ix = np.clip((sx + np.float32(1)) * np.float32(0.5) * (W - 1), 0.0, W - 1).astype(
        np.float32
    )
    iy = np.clip((sy + np.float32(1)) * np.float32(0.5) * (H - 1), 0.0, H - 1).astype(
        np.float32
    )
    ix0f = np.floor(ix)
    iy0f = np.floor(iy)
    wx = (ix - ix0f).astype(np.float32)
    wy = (iy - iy0f).astype(np.float32)
    ix0 = ix0f.astype(np.int64)
    iy0 = iy0f.astype(np.int64)
    ix1 = np.minimum(ix0 + 1, W - 1)
    iy1 = np.minimum(iy0 + 1, H - 1)
    A = np.zeros((HW, HW), dtype=np.float32)
    for r in range(H):
        for cc in range(W):
            p = r * W + cc
            A[p, iy0[r, cc] * W + ix0[r, cc]] += (1 - wy[r, cc]) * (1 - wx[r, cc])
            A[p, iy0[r, cc] * W + ix1[r, cc]] += (1 - wy[r, cc]) * wx[r, cc]
            A[p, iy1[r, cc] * W + ix0[r, cc]] += wy[r, cc] * (1 - wx[r, cc])
            A[p, iy1[r, cc] * W + ix1[r, cc]] += wy[r, cc] * wx[r, cc]
    return A


UNIQ_THETAS, BIN2UID = np.unique(VALID_THETAS, return_inverse=True)
NU = len(UNIQ_THETAS)  # 20
AMATS = np.stack([_rotation_matrix(t) for t in UNIQ_THETAS])  # [NU, 49, 49]


def _split16(a):
    hi = a.astype(np.float16)
    lo = (a.astype(np.float32) - hi.astype(np.float32)).astype(np.float16)
    return hi, lo


def _build_l1_consts():
    # CS64 [49, 64]: col 2j -> rho_j*C_j ; col 2j+1 -> rho_j*S_j (interleaved
    # so Re^2+Im^2 is a segmented pair reduce after transpose)
    cs = np.zeros((HW, 64), dtype=np.float32)
    cs[:, 0 : 2 * NBINS : 2] = (CMAT * VALID_RHOS[:, None]).T
    cs[:, 1 : 2 * NBINS : 2] = (SMAT * VALID_RHOS[:, None]).T
    cs_hi, cs_lo = _split16(cs)
    # term A lhsT [98, 64] = [Clo ; Chi] (matches rhs [Xhi ; Xlo])
    lhsA = np.concatenate([cs_lo, cs_hi], axis=0)
    lhsB = cs_hi  # [49, 64], rhs = Xhi rows
    ident = np.eye(128, dtype=np.float32)
    iota = np.broadcast_to(
        np.tile(np.arange(32, dtype=np.uint32), 16), (128, 16 * 32)
    ).copy()
    return lhsA, lhsB, ident, iota


L1_LHSA, L1_LHSB, L1_IDENT, L1_IOTA = _build_l1_consts()


def _build_l2_consts():
    # per class u: lhsT = Ahi_u^T [49, 49] fp16 (fp16-A error ~2e-4 rel, fine
    # for the output path; the decision path stays fp32-exact).
    # Stored at partition rows 0:49 AND 64:113 so both block-split halves
    # have matching matmul base partitions (0 and 64).
    at = AMATS.transpose(0, 2, 1).astype(np.float16)  # [NU, 49, 49] lhsT
    a = np.concatenate(list(at), axis=1)  # [49, NU*49]
    lhsA = np.zeros((113, NU * HW), dtype=np.float16)
    lhsA[0:49] = a
    lhsA[64:113] = a
    return np.ascontiguousarray(lhsA)


L2_LHSA = _build_l2_consts()
CHUNK = 1024  # apply slots per half-chunk

# ---------------------------------------------------------------------------
# launch 1: decision kernel
# ---------------------------------------------------------------------------


SUPER = 2048  # samples per super-group (4 subtiles of 512)
N_SUPER = PER // SUPER  # 32
STAGE_SUPER = 4  # super-groups per Mx staging DMA -> [128, 64] f32


def build_decision_kernel():
    nc = bacc.Bacc("TRN2", target_bir_lowering=False, debug=False, num_devices=N_CORES)
    xt2 = nc.dram_tensor("xt2", [98, PER], F16, kind="ExternalInput")
    lhsa = nc.dram_tensor("lhsa", [98, 64], F16, kind="ExternalInput")
    lhsb = nc.dram_tensor("lhsb", [49, 64], F16, kind="ExternalInput")
    ident = nc.dram_tensor("ident", [128, 128], F32, kind="ExternalInput")
    iota = nc.dram_tensor("iota", [128, 16 * 32], U32, kind="ExternalInput")
    mx = nc.dram_tensor(
        "mx",
        [N_SUPER // STAGE_SUPER, 128, STAGE_SUPER * 16],
        F32,
        kind="ExternalOutput",
    )

    with tile.TileContext(nc) as tc:
        with (
            tc.tile_pool(name="const", bufs=1) as cpool,
            tc.tile_pool(name="xin", bufs=8) as xpool,
            tc.tile_pool(name="sq", bufs=2) as sqpool,
            tc.tile_pool(name="w", bufs=2) as wpool,
            tc.tile_pool(name="stage", bufs=2) as stpool,
            tc.tile_pool(name="psF", bufs=2, space="PSUM") as psF,
            tc.tile_pool(name="psT", bufs=2, space="PSUM") as psT,
        ):
            c_lhsa = cpool.tile([98, 64], F16)
            c_lhsb = cpool.tile([49, 64], F16)
            c_id = cpool.tile([128, 128], F32)
            c_iota = cpool.tile([128, 16 * 32], U32)
            nc.sync.dma_start(out=c_lhsa, in_=lhsa.ap())
            nc.sync.dma_start(out=c_lhsb, in_=lhsb.ap())
            nc.sync.dma_start(out=c_id, in_=ident.ap())
            nc.sync.dma_start(out=c_iota, in_=iota.ap())

            xt2_ap = xt2.ap()
            for gs in range(N_SUPER // STAGE_SUPER):
                stg = stpool.tile([128, STAGE_SUPER * 16], F32, tag="stage")
                for gi in range(STAGE_SUPER):
                    g = gs * STAGE_SUPER + gi
                    base = g * SUPER
                    # one load for all 4 subtiles [98, 2048] fp16
                    xfull = xpool.tile([98, SUPER], F16, tag="xfull")
                    nc.sync.dma_start(out=xfull, in_=xt2_ap[:, base : base + SUPER])
                    xs = [xfull[:, s * TILE : (s + 1) * TILE] for s in range(4)]
                    # FFT into one psum [128, 1024]; weight-batched:
                    # 4x term A (lhsa), then 4x term B (lhsb)
                    f8 = psF.tile([128, 2 * TILE], F32, tag="f8")
                    quads = [
                        (f8[0:64, 0:TILE], (0, 0), xs[0]),
                        (f8[64:128, 0:TILE], (0, 64), xs[1]),
                        (f8[0:64, TILE : 2 * TILE], (0, 0), xs[2]),
                        (f8[64:128, TILE : 2 * TILE], (0, 64), xs[3]),
                    ]
                    for out_ap, tp, xt_ in quads:
                        nc.tensor.matmul(
                            out_ap, c_lhsa, xt_, start=True, stop=False, tile_position=tp
                        )
                    for out_ap, tp, xt_ in quads:
                        nc.tensor.matmul(
                            out_ap,
                            c_lhsb,
                            xt_[0:49, :],
                            start=False,
                            stop=True,
                            tile_position=tp,
                        )
                    # square (ACT) -> sbuf [128, 1024]
                    fsq = sqpool.tile([128, 2 * TILE], F32, tag="fsq")
                    nc.scalar.activation(
                        out=fsq, in_=f8, func=mybir.ActivationFunctionType.Square
                    )
                    # PE transpose 8 blocks -> psum FsqT [128, 1024]
                    ft = psT.tile([128, 2 * TILE], F32, tag="ft")
                    for b in range(8):
                        nc.tensor.transpose(
                            ft[:, 128 * b : 128 * (b + 1)],
                            fsq[:, 128 * b : 128 * (b + 1)],
                            c_id,
                        )
                    # W = Re^2 + Im^2 : segmented pair reduce [128, 16, 32]
                    wt = wpool.tile([128, 16, 32], F32, tag="wt")
                    nc.vector.tensor_reduce(
                        out=wt,
                        in_=ft.rearrange("p (s b t) -> p s b t", s=16, b=32),
                        axis=mybir.AxisListType.X,
                        op=mybir.AluOpType.add,
                    )
                    # pack bin id into low 5 mantissa bits
                    wm = wpool.tile([128, 16, 32], F32, tag="wm")
                    nc.vector.tensor_scalar(
                        out=wm.bitcast(U32),
                        in0=wt.bitcast(U32),
                        scalar1=0xFFFFFFE0,
                        scalar2=None,
                        op0=mybir.AluOpType.bitwise_and,
                    )
                    wp = wpool.tile([128, 16, 32], F32, tag="wp")
                    nc.vector.tensor_tensor(
                        out=wp.bitcast(U32),
                        in0=wm.bitcast(U32),
                        in1=c_iota.rearrange("p (s c) -> p s c", c=32),
                        op=mybir.AluOpType.bitwise_or,
                    )
                    # segmented max over bins -> [128, 16]
                    nc.vector.tensor_reduce(
                        out=stg[:, gi * 16 : (gi + 1) * 16],
                        in_=wp,
                        axis=mybir.AxisListType.X,
                        op=mybir.AluOpType.max,
                    )
                nc.sync.dma_start(out=mx.ap()[gs], in_=stg)
    nc.compile()
    return nc


def decode_mx(mx_arr):
    """mx [N_SUPER//SS, 128, SS*16] f32 -> k [PER] int (bin index).

    col = gi*16 + seg ; seg = 2*b + s (b in 0..7, s in 0..1)
    sample = (gs*SS+gi)*2048 + (b//4)*1024 + s*512 + 128*(b%4) + i
    """
    bits = np.minimum(mx_arr.view(np.uint32) & np.uint32(31), NBINS - 1)
    k = np.empty(PER, dtype=np.int64)
    gs_, i_, c_ = np.indices(bits.shape)
    gi_ = c_ // 16
    seg = c_ % 16
    b_ = seg // 2
    s_ = seg % 2
    sample = (
        (gs_ * STAGE_SUPER + gi_) * SUPER
        + (b_ // 4) * 1024
        + s_ * 512
        + 128 * (b_ % 4)
        + i_
    )
    k[sample.ravel()] = bits.ravel()
    return k


# ---------------------------------------------------------------------------
# launch 2: apply kernel (schedule depends on per-class capacities)
# ---------------------------------------------------------------------------


def build_apply_kernel(caps):
    """Apply kernel over class-sorted slots, block-split 98-partition layout.

    xs2 [98, CAP2] fp16: rows 0:49 = slots [0, CAP2), rows 49:98 = slots
    [CAP2, 2*CAP2). yst2 same layout, fp16 out. One load + one store per
    chunk pair; K=49 matmul per 512-slot tile.
    """
    cap_total = int(np.sum(caps))
    assert cap_total % (2 * CHUNK) == 0
    cap2 = cap_total // 2
    nc = bacc.Bacc("TRN2", target_bir_lowering=False, debug=False, num_devices=N_CORES)
    xs2 = nc.dram_tensor("xs2", [113, cap2], F16, kind="ExternalInput")
    lhsa = nc.dram_tensor("lhsa", [113, NU * HW], F16, kind="ExternalInput")
    yst2 = nc.dram_tensor("yst2", [113, cap2], F16, kind="ExternalOutput")

    # class of each 512-slot tile
    tile_class = []
    for u in range(NU):
        tile_class += [u] * (int(caps[u]) // TILE)

    def cls(slot):
        return tile_class[slot // TILE]

    with tile.TileContext(nc) as tc:
        with (
            tc.tile_pool(name="const", bufs=1) as cpool,
            tc.tile_pool(name="xin", bufs=3) as xpool,
            tc.tile_pool(name="yout", bufs=3) as ypool,
            tc.tile_pool(name="psY", bufs=2, space="PSUM") as psY,
        ):
            c_lhsa = cpool.tile([113, NU * HW], F16)
            nc.sync.dma_start(out=c_lhsa, in_=lhsa.ap())
            x_ap = xs2.ap()
            y_ap = yst2.ap()
            for c0 in range(0, cap2, CHUNK):
                xt = xpool.tile([113, CHUNK], F16, tag="xt")
                nc.sync.dma_start(out=xt, in_=x_ap[:, c0 : c0 + CHUNK])
                psa = psY.tile([49, CHUNK], F32, tag="psa")
                psb = psY.tile([49, CHUNK], F32, tag="psb")
                for t in range(CHUNK // TILE):
                    ua = cls(c0 + t * TILE)
                    ub = cls(cap2 + c0 + t * TILE)
                    nc.tensor.matmul(
                        psa[:, t * TILE : (t + 1) * TILE],
                        c_lhsa[0:49, ua * HW : (ua + 1) * HW],
                        xt[0:49, t * TILE : (t + 1) * TILE],
                        start=True,
                        stop=True,
                    )
                    nc.tensor.matmul(
                        psb[:, t * TILE : (t + 1) * TILE],
                        c_lhsa[64:113, ub * HW : (ub + 1) * HW],
                        xt[64:113, t * TILE : (t + 1) * TILE],
                        start=True,
                        stop=True,
                    )
                yt = ypool.tile([113, CHUNK], F16, tag="yt")
                nc.scalar.copy(yt[0:49, :], psa)
                nc.vector.tensor_copy(yt[64:113, :], psb)
                if (c0 // CHUNK) % 2 == 0:
                    nc.sync.dma_start(out=y_ap[:, c0 : c0 + CHUNK], in_=yt)
                else:
                    nc.scalar.dma_start(out=y_ap[:, c0 : c0 + CHUNK], in_=yt)
    nc.compile()
    return nc


# ---------------------------------------------------------------------------
# host orchestration
# ---------------------------------------------------------------------------


def kernel(x: np.ndarray) -> tuple[np.ndarray, np.ndarray]:
    x = np.asarray(x)
    assert x.shape == (B, 1, H, W) and x.dtype == np.float32
    xf = np.ascontiguousarray(x.reshape(B, HW))
    shards = xf.reshape(N_CORES, PER, HW)

    xhi = shards.astype(np.float16)
    xlo = (shards - xhi.astype(np.float32)).astype(np.float16)

    # ---- launch 1 ----
    nc1 = build_decision_kernel()
    in_maps = []
    for c in range(N_CORES):
        xt2 = np.concatenate(
            [np.ascontiguousarray(xhi[c].T), np.ascontiguousarray(xlo[c].T)], axis=0
        )  # [98, PER] hi rows first
        in_maps.append(
            {
                "xt2": xt2,
                "lhsa": L1_LHSA.astype(np.float16),
                "lhsb": L1_LHSB.astype(np.float16),
                "ident": L1_IDENT,
                "iota": L1_IOTA,
            }
        )
    res1 = run_bass_kernel_spmd(
        nc1, in_maps, core_ids=list(range(N_CORES)), trace=_TRACE
    )
    LAST_EXEC_NS["decision"] = res1.exec_time_ns
    k_all = np.stack([decode_mx(res1.results[c]["mx"]) for c in range(N_CORES)])

    theta = VALID_THETAS[k_all.reshape(B)]

    # ---- host glue: class-sorted layout ----
    uid = BIN2UID[k_all]  # [N_CORES, PER]
    counts = np.stack([np.bincount(uid[c], minlength=NU) for c in range(N_CORES)])
    caps = ((counts.max(axis=0) + TILE - 1) // TILE) * TILE
    # pad total capacity to a multiple of 2*CHUNK (pad tiles run class 0)
    pad = (-int(caps.sum())) % (2 * CHUNK)
    caps[NU - 1] += pad
    cap_off = np.concatenate([[0], np.cumsum(caps)])[:NU]
    cap_total = int(caps.sum())
    cap2 = cap_total // 2

    slots = np.empty((N_CORES, PER), dtype=np.int64)
    for c in range(N_CORES):
        order = np.argsort(uid[c], kind="stable")
        rank = np.empty(PER, dtype=np.int64)
        rank[order] = np.arange(PER)
        cum = np.concatenate([[0], np.cumsum(counts[c])])[:NU]
        slots[c] = cap_off[uid[c]] + (rank - cum[uid[c]])

    # ---- launch 2 ----
    nc2 = build_apply_kernel(caps)
    in_maps2 = []
    for c in range(N_CORES):
        xs2 = np.zeros((113, cap2), dtype=np.float16)
        lo_half = slots[c] < cap2
        sl = slots[c][lo_half]
        xs2[0:49, sl] = xhi[c][lo_half].T
        sh = slots[c][~lo_half] - cap2
        xs2[64:113, sh] = xhi[c][~lo_half].T
        in_maps2.append({"xs2": xs2, "lhsa": L2_LHSA})
    res2 = run_bass_kernel_spmd(
        nc2, in_maps2, core_ids=list(range(N_CORES)), trace=_TRACE
    )
    LAST_EXEC_NS["apply"] = res2.exec_time_ns

    out = np.empty((N_CORES, PER, HW), dtype=np.float32)
    for c in range(N_CORES):
        yst2 = res2.results[c]["yst2"]  # [113, cap2] fp16
        yflat = np.concatenate([yst2[0:49, :].T, yst2[64:113, :].T], axis=0)
        out[c] = yflat[slots[c]].astype(np.float32)

    return out.reshape(B, 1, H, W), theta


# revision 23
# speedup vs baseline: 1.0131x; 1.0131x over previous
"""Trainium2 Bass kernel for nn_FAA (Fourier-argmax alignment).

Per sample (7x7 image): rfft2 magnitudes -> argmax over 27 frequency bins
(weighted by rho) -> rotation angle theta -> bilinear grid-sample rotation.

Device pipeline (2 launches, 8 cores, batch sharded):
  Launch 1 (decision): FFT as matmul (fp16 hi/lo, fp32-accurate), square,
    |F|^2*rho^2 per bin, argmax via mantissa-packed segmented max-reduce.
  Host glue: bin -> unique-angle class, class-sort layout (counting sort).
  Launch 2 (apply): per-class rotation matrices as dense matmuls over the
    class-sorted stream (fp16 hi/lo, fp32-accurate).
"""

import os
import sys

import numpy as np

sys.path.insert(0, "/opt/trn_rl_repo")

import concourse.bacc as bacc
import concourse.tile as tile
from concourse import mybir
from concourse.bass_utils import run_bass_kernel_spmd

F32 = mybir.dt.float32
F16 = mybir.dt.float16
U32 = mybir.dt.uint32

H = W = 7
HW = 49
B = 524288
N_CORES = 8
PER = B // N_CORES  # 65536
GRP = 1024  # samples per launch-1 group (2 subtiles of 512)
N_GRP = PER // GRP  # 64
STAGE_GROUPS = 8  # groups per Mx staging DMA
TILE = 512  # samples per launch-2 matmul tile

LAST_EXEC_NS = {"decision": None, "apply": None}
_TRACE = os.environ.get("FAA_TRACE", "0") == "1"

# ---------------------------------------------------------------------------
# constants (pure math, replicates the reference module exactly)
# ---------------------------------------------------------------------------


def _freq_constants():
    h_shift = np.roll(np.arange(H) - H // 2, H // 2)
    w_shift = np.concatenate([np.arange(W // 2 + 1)[: W // 2], np.array([-(W // 2)])])
    y, xg = np.meshgrid(h_shift, w_shift, indexing="ij")
    rho = np.sqrt((xg**2 + y**2).astype(np.float32))
    theta = np.arctan2(y.astype(np.float32), xg.astype(np.float32))
    theta = (theta + 2.0 * np.pi) % (2.0 * np.pi)
    valid = np.flatnonzero((rho > 1e-8).ravel())
    return valid, theta.ravel()[valid].astype(np.float32), rho.ravel()[valid].astype(
        np.float32
    )


VALID_IDX, VALID_THETAS, VALID_RHOS = _freq_constants()
NBINS = len(VALID_IDX)  # 27


def _fourier_mats():
    C = np.zeros((NBINS, HW), dtype=np.float64)
    S = np.zeros((NBINS, HW), dtype=np.float64)
    for j, flat in enumerate(VALID_IDX):
        u, v = divmod(int(flat), W // 2 + 1)
        for h in range(H):
            for w in range(W):
                ang = 2.0 * np.pi * (u * h + v * w) / 7.0
                C[j, h * W + w] = np.cos(ang) / 7.0
                S[j, h * W + w] = -np.sin(ang) / 7.0
    return C.astype(np.float32), S.astype(np.float32)


CMAT, SMAT = _fourier_mats()

_ys = np.linspace(-1.0, 1.0, H, dtype=np.float32)
_xs = np.linspace(-1.0, 1.0, W, dtype=np.float32)
GY, GX = np.meshgrid(_ys, _xs, indexing="ij")[0], np.meshgrid(_ys, _xs, indexing="ij")[
    1
]


def _rotation_matrix(theta):
    c = np.float32(np.cos(theta))
    s = np.float32(np.sin(theta))
    sx = GX * c - GY * s
    sy = GX * s + GY * c
    ix = np.clip((sx + np.float32(1)) * np.float32(0.5) * (W - 1), 0.0, W - 1).astype(
        np.float32
    )
    iy = np.clip((sy + np.float32(1)) * np.float32(0.5) * (H - 1), 0.0, H - 1).astype(
        np.float32
    )
    ix0f = np.floor(ix)
    iy0f = np.floor(iy)
    wx = (ix - ix0f).astype(np.float32)
    wy = (iy - iy0f).astype(np.float32)
    ix0 = ix0f.astype(np.int64)
    iy0 = iy0f.astype(np.int64)
    ix1 = np.minimum(ix0 + 1, W - 1)
    iy1 = np.minimum(iy0 + 1, H - 1)
    A = np.zeros((HW, HW), dtype=np.float32)
    for r in range(H):
        for cc in range(W):
            p = r * W + cc
            A[p, iy0[r, cc] * W + ix0[r, cc]] += (1 - wy[r, cc]) * (1 - wx[r, cc])
            A[p, iy0[r, cc] * W + ix1[r, cc]] += (1 - wy[r, cc]) * wx[r, cc]
            A[p, iy1[r, cc] * W + ix0[r, cc]] += wy[r, cc] * (1 - wx[r, cc])
            A[p, iy1[r, cc] * W + ix1[r, cc]] += wy[r, cc] * wx[r, cc]
    return A


UNIQ_THETAS, BIN2UID = np.unique(VALID_THETAS, return_inverse=True)
NU = len(UNIQ_THETAS)  # 20
AMATS = np.stack([_rotation_matrix(t) for t in UNIQ_THETAS])  # [NU, 49, 49]


def _split16(a):
    hi = a.astype(np.float16)
    lo = (a.astype(np.float32) - hi.astype(np.float32)).astype(np.float16)
    return hi, lo


def _build_l1_consts():
    # CS64 [49, 64]: col 2j -> rho_j*C_j ; col 2j+1 -> rho_j*S_j (interleaved
    # so Re^2+Im^2 is a segmented pair reduce after transpose)
    cs = np.zeros((HW, 64), dtype=np.float32)
    cs[:, 0 : 2 * NBINS : 2] = (CMAT * VALID_RHOS[:, None]).T
    cs[:, 1 : 2 * NBINS : 2] = (SMAT * VALID_RHOS[:, None]).T
    cs_hi, cs_lo = _split16(cs)
    # term A lhsT [98, 64] = [Clo ; Chi] (matches rhs [Xhi ; Xlo])
    lhsA = np.concatenate([cs_lo, cs_hi], axis=0)
    lhsB = cs_hi  # [49, 64], rhs = Xhi rows
    ident = np.eye(128, dtype=np.float32)
    iota = np.broadcast_to(
        np.tile(np.arange(32, dtype=np.uint32), 16), (128, 16 * 32)
    ).copy()
    return lhsA, lhsB, ident, iota


L1_LHSA, L1_LHSB, L1_IDENT, L1_IOTA = _build_l1_consts()


def _build_l2_consts():
    # per class u: lhsT = Ahi_u^T [49, 49] fp16 (fp16-A error ~2e-4 rel, fine
    # for the output path; the decision path stays fp32-exact).
    # Stored at partition rows 0:49 AND 64:113 so both block-split halves
    # have matching matmul base partitions (0 and 64).
    at = AMATS.transpose(0, 2, 1).astype(np.float16)  # [NU, 49, 49] lhsT
    a = np.concatenate(list(at), axis=1)  # [49, NU*49]
    lhsA = np.zeros((113, NU * HW), dtype=np.float16)
    lhsA[0:49] = a
    lhsA[64:113] = a
    return np.ascontiguousarray(lhsA)


L2_LHSA = _build_l2_consts()
CHUNK = 1024  # apply slots per half-chunk

# ---------------------------------------------------------------------------
# launch 1: decision kernel
# ---------------------------------------------------------------------------


SUPER = 2048  # samples per super-group (4 subtiles of 512)
N_SUPER = PER // SUPER  # 32
STAGE_SUPER = 4  # super-groups per Mx staging DMA -> [128, 64] f32


def build_decision_kernel():
    nc = bacc.Bacc("TRN2", target_bir_lowering=False, debug=False, num_devices=N_CORES)
    xt2 = nc.dram_tensor("xt2", [98, PER], F16, kind="ExternalInput")
    lhsa = nc.dram_tensor("lhsa", [98, 64], F16, kind="ExternalInput")
    lhsb = nc.dram_tensor("lhsb", [49, 64], F16, kind="ExternalInput")
    ident = nc.dram_tensor("ident", [128, 128], F32, kind="ExternalInput")
    iota = nc.dram_tensor("iota", [128, 16 * 32], U32, kind="ExternalInput")
    mx = nc.dram_tensor(
        "mx",
        [N_SUPER // STAGE_SUPER, 128, STAGE_SUPER * 16],
        F32,
        kind="ExternalOutput",
    )

    with tile.TileContext(nc) as tc:
        with (
            tc.tile_pool(name="const", bufs=1) as cpool,
            tc.tile_pool(name="xin", bufs=8) as xpool,
            tc.tile_pool(name="sq", bufs=2) as sqpool,
            tc.tile_pool(name="w", bufs=2) as wpool,
            tc.tile_pool(name="stage", bufs=2) as stpool,
            tc.tile_pool(name="psF", bufs=2, space="PSUM") as psF,
            tc.tile_pool(name="psT", bufs=2, space="PSUM") as psT,
        ):
            c_lhsa = cpool.tile([98, 64], F16)
            c_lhsb = cpool.tile([49, 64], F16)
            c_id = cpool.tile([128, 128], F32)
            c_iota = cpool.tile([128, 16 * 32], U32)
            nc.sync.dma_start(out=c_lhsa, in_=lhsa.ap())
            nc.sync.dma_start(out=c_lhsb, in_=lhsb.ap())
            nc.sync.dma_start(out=c_id, in_=ident.ap())
            nc.sync.dma_start(out=c_iota, in_=iota.ap())

            xt2_ap = xt2.ap()
            for gs in range(N_SUPER // STAGE_SUPER):
                stg = stpool.tile([128, STAGE_SUPER * 16], F32, tag="stage")
                for gi in range(STAGE_SUPER):
                    g = gs * STAGE_SUPER + gi
                    base = g * SUPER
                    # one load for all 4 subtiles [98, 2048] fp16
                    xfull = xpool.tile([98, SUPER], F16, tag="xfull")
                    nc.sync.dma_start(out=xfull, in_=xt2_ap[:, base : base + SUPER])
                    xs = [xfull[:, s * TILE : (s + 1) * TILE] for s in range(4)]
                    # FFT into one psum [128, 1024]; weight-batched:
                    # 4x term A (lhsa), then 4x term B (lhsb)
                    f8 = psF.tile([128, 2 * TILE], F32, tag="f8")
                    quads = [
                        (f8[0:64, 0:TILE], (0, 0), xs[0]),
                        (f8[64:128, 0:TILE], (0, 64), xs[1]),
                        (f8[0:64, TILE : 2 * TILE], (0, 0), xs[2]),
                        (f8[64:128, TILE : 2 * TILE], (0, 64), xs[3]),
                    ]
                    for out_ap, tp, xt_ in quads:
                        nc.tensor.matmul(
                            out_ap, c_lhsa, xt_, start=True, stop=False, tile_position=tp
                        )
                    for out_ap, tp, xt_ in quads:
                        nc.tensor.matmul(
                            out_ap,
                            c_lhsb,
                            xt_[0:49, :],
                            start=False,
                            stop=True,
                            tile_position=tp,
                        )
                    # square (ACT) -> sbuf [128, 1024]
                    fsq = sqpool.tile([128, 2 * TILE], F32, tag="fsq")
                    nc.scalar.activation(
                        out=fsq, in_=f8, func=mybir.ActivationFunctionType.Square
                    )
                    # PE transpose 8 blocks -> psum FsqT [128, 1024]
                    ft = psT.tile([128, 2 * TILE], F32, tag="ft")
                    for b in range(8):
                        nc.tensor.transpose(
                            ft[:, 128 * b : 128 * (b + 1)],
                            fsq[:, 128 * b : 128 * (b + 1)],
                            c_id,
                        )
                    # W = Re^2 + Im^2 : segmented pair reduce [128, 16, 32]
                    wt = wpool.tile([128, 16, 32], F32, tag="wt")
                    nc.vector.tensor_reduce(
                        out=wt,
                        in_=ft.rearrange("p (s b t) -> p s b t", s=16, b=32),
                        axis=mybir.AxisListType.X,
                        op=mybir.AluOpType.add,
                    )
                    # pack bin id into low 5 mantissa bits
                    wm = wpool.tile([128, 16, 32], F32, tag="wm")
                    nc.vector.tensor_scalar(
                        out=wm.bitcast(U32),
                        in0=wt.bitcast(U32),
                        scalar1=0xFFFFFFE0,
                        scalar2=None,
                        op0=mybir.AluOpType.bitwise_and,
                    )
                    wp = wpool.tile([128, 16, 32], F32, tag="wp")
                    nc.vector.tensor_tensor(
                        out=wp.bitcast(U32),
                        in0=wm.bitcast(U32),
                        in1=c_iota.rearrange("p (s c) -> p s c", c=32),
                        op=mybir.AluOpType.bitwise_or,
                    )
                    # segmented max over bins -> [128, 16]
                    nc.vector.tensor_reduce(
                        out=stg[:, gi * 16 : (gi + 1) * 16],
                        in_=wp,
                        axis=mybir.AxisListType.X,
                        op=mybir.AluOpType.max,
                    )
                nc.sync.dma_start(out=mx.ap()[gs], in_=stg)
    nc.compile()
    return nc


def decode_mx(mx_arr):
    """mx [N_SUPER//SS, 128, SS*16] f32 -> k [PER] int (bin index).

    col = gi*16 + seg ; seg = 2*b + s (b in 0..7, s in 0..1)
    sample = (gs*SS+gi)*2048 + (b//4)*1024 + s*512 + 128*(b%4) + i
    """
    bits = np.minimum(mx_arr.view(np.uint32) & np.uint32(31), NBINS - 1)
    k = np.empty(PER, dtype=np.int64)
    gs_, i_, c_ = np.indices(bits.shape)
    gi_ = c_ // 16
    seg = c_ % 16
    b_ = seg // 2
    s_ = seg % 2
    sample = (
        (gs_ * STAGE_SUPER + gi_) * SUPER
        + (b_ // 4) * 1024
        + s_ * 512
        + 128 * (b_ % 4)
        + i_
    )
    k[sample.ravel()] = bits.ravel()
    return k


# ---------------------------------------------------------------------------
# launch 2: apply kernel (schedule depends on per-class capacities)
# ---------------------------------------------------------------------------


def build_apply_kernel(caps):
    """Apply kernel over class-sorted slots, block-split 98-partition layout.

    xs2 [98, CAP2] fp16: rows 0:49 = slots [0, CAP2), rows 49:98 = slots
    [CAP2, 2*CAP2). yst2 same layout, fp16 out. One load + one store per
    chunk pair; K=49 matmul per 512-slot tile.
    """
    cap_total = int(np.sum(caps))
    assert cap_total % (8 * CHUNK) == 0
    cap2 = cap_total // 2
    nc = bacc.Bacc("TRN2", target_bir_lowering=False, debug=False, num_devices=N_CORES)
    xs2 = nc.dram_tensor("xs2", [113, cap2], F16, kind="ExternalInput")
    lhsa = nc.dram_tensor("lhsa", [113, NU * HW], F16, kind="ExternalInput")
    yst2 = nc.dram_tensor("yst2", [113, cap2], F16, kind="ExternalOutput")

    # class of each 512-slot tile
    tile_class = []
    for u in range(NU):
        tile_class += [u] * (int(caps[u]) // TILE)

    def cls(slot):
        return tile_class[slot // TILE]

    with tile.TileContext(nc) as tc:
        with (
            tc.tile_pool(name="const", bufs=1) as cpool,
            tc.tile_pool(name="xin", bufs=3) as xpool,
            tc.tile_pool(name="yout", bufs=3) as ypool,
            tc.tile_pool(name="psY", bufs=2, space="PSUM") as psY,
        ):
            c_lhsa = cpool.tile([113, NU * HW], F16)
            nc.sync.dma_start(out=c_lhsa, in_=lhsa.ap())
            x_ap = xs2.ap()
            y_ap = yst2.ap()
            SPAN = 4 * CHUNK  # slots per DMA (8KB per partition row)
            for s0 in range(0, cap2, SPAN):
                xt = xpool.tile([113, SPAN], F16, tag="xt")
                nc.sync.dma_start(out=xt, in_=x_ap[:, s0 : s0 + SPAN])
                yt = ypool.tile([113, SPAN], F16, tag="yt")
                for ci in range(SPAN // CHUNK):
                    c0 = s0 + ci * CHUNK
                    xoff = ci * CHUNK
                    psa = psY.tile([49, CHUNK], F32, tag="psa")
                    psb = psY.tile([49, CHUNK], F32, tag="psb")
                    for t in range(CHUNK // TILE):
                        ua = cls(c0 + t * TILE)
                        ub = cls(cap2 + c0 + t * TILE)
                        nc.tensor.matmul(
                            psa[:, t * TILE : (t + 1) * TILE],
                            c_lhsa[0:49, ua * HW : (ua + 1) * HW],
                            xt[0:49, xoff + t * TILE : xoff + (t + 1) * TILE],
                            start=True,
                            stop=True,
                        )
                        nc.tensor.matmul(
                            psb[:, t * TILE : (t + 1) * TILE],
                            c_lhsa[64:113, ub * HW : (ub + 1) * HW],
                            xt[64:113, xoff + t * TILE : xoff + (t + 1) * TILE],
                            start=True,
                            stop=True,
                        )
                    nc.scalar.copy(yt[0:49, xoff : xoff + CHUNK], psa)
                    nc.vector.tensor_copy(yt[64:113, xoff : xoff + CHUNK], psb)
                if (s0 // SPAN) % 2 == 0:
                    nc.sync.dma_start(out=y_ap[:, s0 : s0 + SPAN], in_=yt)
                else:
                    nc.scalar.dma_start(out=y_ap[:, s0 : s0 + SPAN], in_=yt)
    nc.compile()
    return nc


# ---------------------------------------------------------------------------
# host orchestration
# ---------------------------------------------------------------------------


def kernel(x: np.ndarray) -> tuple[np.ndarray, np.ndarray]:
    x = np.asarray(x)
    assert x.shape == (B, 1, H, W) and x.dtype == np.float32
    xf = np.ascontiguousarray(x.reshape(B, HW))
    shards = xf.reshape(N_CORES, PER, HW)

    xhi = shards.astype(np.float16)
    xlo = (shards - xhi.astype(np.float32)).astype(np.float16)

    # ---- launch 1 ----
    nc1 = build_decision_kernel()
    in_maps = []
    for c in range(N_CORES):
        xt2 = np.concatenate(
            [np.ascontiguousarray(xhi[c].T), np.ascontiguousarray(xlo[c].T)], axis=0
        )  # [98, PER] hi rows first
        in_maps.append(
            {
                "xt2": xt2,
                "lhsa": L1_LHSA.astype(np.float16),
                "lhsb": L1_LHSB.astype(np.float16),
                "ident": L1_IDENT,
                "iota": L1_IOTA,
            }
        )
    res1 = run_bass_kernel_spmd(
        nc1, in_maps, core_ids=list(range(N_CORES)), trace=_TRACE
    )
    LAST_EXEC_NS["decision"] = res1.exec_time_ns
    k_all = np.stack([decode_mx(res1.results[c]["mx"]) for c in range(N_CORES)])

    theta = VALID_THETAS[k_all.reshape(B)]

    # ---- host glue: class-sorted layout ----
    uid = BIN2UID[k_all]  # [N_CORES, PER]
    counts = np.stack([np.bincount(uid[c], minlength=NU) for c in range(N_CORES)])
    caps = ((counts.max(axis=0) + TILE - 1) // TILE) * TILE
    # pad total capacity to a multiple of 2*CHUNK (pad tiles run class 0)
    pad = (-int(caps.sum())) % (8 * CHUNK)
    caps[NU - 1] += pad
    cap_off = np.concatenate([[0], np.cumsum(caps)])[:NU]
    cap_total = int(caps.sum())
    cap2 = cap_total // 2

    slots = np.empty((N_CORES, PER), dtype=np.int64)
    for c in range(N_CORES):
        order = np.argsort(uid[c], kind="stable")
        rank = np.empty(PER, dtype=np.int64)
        rank[order] = np.arange(PER)
        cum = np.concatenate([[0], np.cumsum(counts[c])])[:NU]
        slots[c] = cap_off[uid[c]] + (rank - cum[uid[c]])

    # ---- launch 2 ----
    nc2 = build_apply_kernel(caps)
    in_maps2 = []
    for c in range(N_CORES):
        xs2 = np.zeros((113, cap2), dtype=np.float16)
        lo_half = slots[c] < cap2
        sl = slots[c][lo_half]
        xs2[0:49, sl] = xhi[c][lo_half].T
        sh = slots[c][~lo_half] - cap2
        xs2[64:113, sh] = xhi[c][~lo_half].T
        in_maps2.append({"xs2": xs2, "lhsa": L2_LHSA})
    res2 = run_bass_kernel_spmd(
        nc2, in_maps2, core_ids=list(range(N_CORES)), trace=_TRACE
    )
    LAST_EXEC_NS["apply"] = res2.exec_time_ns

    out = np.empty((N_CORES, PER, HW), dtype=np.float32)
    for c in range(N_CORES):
        yst2 = res2.results[c]["yst2"]  # [113, cap2] fp16
        yflat = np.concatenate([yst2[0:49, :].T, yst2[64:113, :].T], axis=0)
        out[c] = yflat[slots[c]].astype(np.float32)

    return out.reshape(B, 1, H, W), theta
